# revision 2
# baseline (speedup 1.0000x reference)
"""Trainium2 Bass kernel for masked tanh-clipped attention softmax.

Reference computation (B=16, NQ=NK=2048, KD=QD=KQ=256, CLIP=10):
    k = k_inputs @ Wk                     [B, NK, 256]
    q = q_inputs @ Wq                     [B, NQ, 256]
    s = (q @ k^T) / 16                    [B, NQ, NK]
    s = tanh(s) * 10
    s = where(adjancy, s, -inf)
    out = softmax(s, axis=2)

Kernel strategy (per NeuronCore, 2 batches each across 8 cores):
  * Fold the projections: A = Wq @ Wk^T (256x256), so s = q_in @ A @ k_in^T.
    Computed once on device in fp32.
  * Host passes q_in/k_in pre-transposed to [d, token] layout (the PE
    contracts over the partition dim), plus Wq^T/Wk^T for the A matmul.
  * qaT = A^T @ q_inT per batch in fp32 (PE), rounded to f32r on copy-out.
  * scores = qaT^T @ k_inT in f32r (full PE speed, ~1e-4 rel err).
  * t = tanh(scores/16) on ACT (PSUM -> SBUF).
  * t += adjancy (int32 0/1) on GpSimd.
  * e = exp(10t - 10) on ACT with fused row-sum (accum_out).  Unmasked
    entries give exp(10*tanh) exactly; masked ones are suppressed by e^-10,
    contributing < 5e-5 relative to the row sum (reference gives exact 0).
    No max-subtraction needed: scores are clipped to [-10, 10].
  * out = e * (1/rowsum) on DVE (per-partition tensor_scalar).
"""
import numpy as np

import concourse.bacc as bacc
import concourse.mybir as mybir
from concourse.tile import TileContext
from concourse.bass_utils import run_bass_kernel_spmd

F32 = mybir.dt.float32
F32R = mybir.dt.float32r
I32 = mybir.dt.int32
AF = mybir.ActivationFunctionType

B, NQ, NK = 16, 2048, 2048
D = 256                 # KD = QD = KQ
CORES = 8
BPC = B // CORES        # batches per core
MT = 128                # query rows per tile
NMT = NQ // MT          # 16 m-tiles per batch
CH = 512                # psum bank free-dim (fp32)
NCH = NK // CH          # 4 n-chunks per scores row


def build():
    nc = bacc.Bacc(None, target_bir_lowering=False)

    qT = nc.dram_tensor("qT", [BPC, D, NQ], F32, kind="ExternalInput")
    kT = nc.dram_tensor("kT", [BPC, D, NK], F32, kind="ExternalInput")
    adj = nc.dram_tensor("adj", [BPC, NQ, NK], I32, kind="ExternalInput")
    wqT = nc.dram_tensor("wqT", [D, D], F32, kind="ExternalInput")
    wkT = nc.dram_tensor("wkT", [D, D], F32, kind="ExternalInput")
    out = nc.dram_tensor("out", [BPC, NQ, NK], F32, kind="ExternalOutput")

    with TileContext(nc) as tc:
        with (
            tc.tile_pool(name="const", bufs=1) as cp,
            tc.tile_pool(name="batch", bufs=2) as bp,
            tc.tile_pool(name="mt", bufs=4) as mp,
            tc.tile_pool(name="ps", bufs=2, space="PSUM") as ps,
        ):
            # ---- A = Wq @ Wk^T, once ----
            wq_t = cp.tile([128, 2, D], F32)   # [e-part, e-chunk, d]
            wk_t = cp.tile([128, 2, D], F32)
            nc.sync.dma_start(out=wq_t[:], in_=wqT[:, :].rearrange("(c p) d -> p c d", p=128))
            nc.sync.dma_start(out=wk_t[:], in_=wkT[:, :].rearrange("(c p) d -> p c d", p=128))
            a_sb = []                          # a_sb[dc][:, dp] = A[dc*128+:, :]
            for dc in range(2):
                a_ps = ps.tile([128, D], F32, tag="sc", name=f"a_ps{dc}")
                for ec in range(2):
                    nc.tensor.matmul(
                        a_ps[:],
                        wq_t[:, ec, dc * 128:(dc + 1) * 128],
                        wk_t[:, ec, :],
                        start=(ec == 0),
                        stop=(ec == 1),
                    )
                a_t = cp.tile([128, D], F32, name=f"a_t{dc}")
                nc.vector.tensor_copy(a_t[:], a_ps[:])
                a_sb.append(a_t)

            ebias = cp.tile([128, 1], F32)
            nc.vector.memset(ebias[:], -10.0)

            for b in range(BPC):
                # ---- load this batch's operands ----
                qT_t = bp.tile([128, 2, NQ], F32)
                kT_t = bp.tile([128, 2, NK], F32, bufs=1)   # staging, consumed by the round copy
                nc.sync.dma_start(out=qT_t[:], in_=qT[b].rearrange("(c p) n -> p c n", p=128))
                nc.sync.dma_start(out=kT_t[:], in_=kT[b].rearrange("(c p) n -> p c n", p=128))
                kTr_t = bp.tile([128, 2, NK], F32R)
                nc.vector.tensor_copy(kTr_t[:], kT_t[:])    # round to f32r
                kTr = kTr_t[:]

                # ---- qaT[d', m] = sum_d A[d, d'] q_inT[d, m]  (fp32) ----
                qa_t = bp.tile([128, 2, NQ], F32R)
                for dp in range(2):
                    for mc in range(NCH):
                        qa_ps = ps.tile([128, CH], F32, tag="sc", name="qa_ps")
                        for dc in range(2):
                            nc.tensor.matmul(
                                qa_ps[:],
                                a_sb[dc][:, dp * 128:(dp + 1) * 128],
                                qT_t[:, dc, mc * CH:(mc + 1) * CH],
                                start=(dc == 0),
                                stop=(dc == 1),
                            )
                        nc.vector.tensor_copy(qa_t[:, dp, mc * CH:(mc + 1) * CH], qa_ps[:])

                # ---- per m-tile: scores -> tanh -> +adj -> exp+sum -> norm ----
                for mt in range(NMT):
                    sc_ps = ps.tile([128, NK], F32, tag="sc", name="sc_ps")
                    for dp in range(2):
                        for n in range(NCH):
                            nc.tensor.matmul(
                                sc_ps[:, n * CH:(n + 1) * CH],
                                qa_t[:, dp, mt * MT:(mt + 1) * MT],
                                kTr[:, dp, n * CH:(n + 1) * CH],
                                start=(dp == 0),
                                stop=(dp == 1),
                            )
                    adj_t = mp.tile([128, NK], I32)
                    nc.sync.dma_start(out=adj_t[:], in_=adj[b, mt * MT:(mt + 1) * MT, :])
                    t_t = mp.tile([128, NK], F32)
                    nc.scalar.activation(t_t[:], sc_ps[:], AF.Tanh, scale=1.0 / 16.0)
                    nc.gpsimd.tensor_add(t_t[:], t_t[:], adj_t[:])
                    rsum = mp.tile([128, 1], F32, bufs=2)
                    nc.scalar.activation(t_t[:], t_t[:], AF.Exp, bias=ebias[:], scale=10.0, accum_out=rsum[:])
                    rcp = mp.tile([128, 1], F32, bufs=2)
                    nc.vector.reciprocal(rcp[:], rsum[:])
                    nc.vector.tensor_scalar_mul(t_t[:], t_t[:], rcp[:])
                    nc.sync.dma_start(out=out[b, mt * MT:(mt + 1) * MT, :], in_=t_t[:])
    nc.compile()
    return nc


_NC = None


def _get_nc():
    global _NC
    if _NC is None:
        _NC = build()
    return _NC


def kernel(k_inputs, q_inputs, adjancy, Wk, Wq):
    nc = _get_nc()
    wqT = np.ascontiguousarray(Wq.T)
    wkT = np.ascontiguousarray(Wk.T)
    in_maps = []
    for c in range(CORES):
        lo, hi = c * BPC, (c + 1) * BPC
        in_maps.append({
            "qT": np.ascontiguousarray(q_inputs[lo:hi].transpose(0, 2, 1)),
            "kT": np.ascontiguousarray(k_inputs[lo:hi].transpose(0, 2, 1)),
            "adj": np.ascontiguousarray(adjancy[lo:hi]),
            "wqT": wqT,
            "wkT": wkT,
        })
    res = run_bass_kernel_spmd(nc, in_maps, core_ids=list(range(CORES)))
    return np.concatenate([res.results[c]["out"] for c in range(CORES)], axis=0)


# revision 9
# speedup vs baseline: 2.1758x; 2.1758x over previous
"""Trainium2 Bass kernel for masked tanh-clipped attention softmax.

Reference computation (B=16, NQ=NK=2048, KD=QD=KQ=256, CLIP=10):
    k = k_inputs @ Wk                     [B, NK, 256]
    q = q_inputs @ Wq                     [B, NQ, 256]
    s = (q @ k^T) / 16                    [B, NQ, NK]
    s = tanh(s) * 10
    s = where(adjancy, s, -inf)
    out = softmax(s, axis=2)

Kernel strategy (per NeuronCore, 2 batches each across 8 cores):
  * Fold the projections: A = Wq @ Wk^T (256x256), so s = q_in @ A @ k_in^T.
    Computed once on device in fp32.
  * Host passes q_in/k_in pre-transposed to [d, token] layout (the PE
    contracts over the partition dim), plus Wq^T/Wk^T for the A matmul.
  * qaT = A^T @ q_inT per batch in fp32 (PE), rounded to f32r on copy-out.
  * scores = qaT^T @ k_inT in f32r (full PE speed, ~1e-4 rel err).
  * t = tanh(scores/16) on ACT (PSUM -> SBUF).
  * t += adjancy (int32 0/1) on GpSimd.
  * e = exp(10t - 10) on ACT with fused row-sum (accum_out).  Unmasked
    entries give exp(10*tanh) exactly; masked ones are suppressed by e^-10,
    contributing < 5e-5 relative to the row sum (reference gives exact 0).
    No max-subtraction needed: scores are clipped to [-10, 10].
  * out = e * (1/rowsum) on DVE (per-partition tensor_scalar).
"""
import numpy as np

import concourse.bacc as bacc
import concourse.mybir as mybir
from concourse.tile import TileContext
from concourse.bass_utils import run_bass_kernel_spmd

F32 = mybir.dt.float32
F32R = mybir.dt.float32r
I32 = mybir.dt.int32
AF = mybir.ActivationFunctionType

B, NQ, NK = 16, 2048, 2048
D = 256                 # KD = QD = KQ
CORES = 8
BPC = B // CORES        # batches per core
MT = 128                # query rows per tile
NMT = NQ // MT          # 16 m-tiles per batch
CH = 512                # psum bank free-dim (fp32)
NCH = NK // CH          # 4 n-chunks per scores row


def build(reps=1):
    nc = bacc.Bacc(None, target_bir_lowering=False)

    qT = nc.dram_tensor("qT", [BPC, D, NQ], F32, kind="ExternalInput")
    kT = nc.dram_tensor("kT", [BPC, D, NK], F32, kind="ExternalInput")
    adj = nc.dram_tensor("adj", [BPC, NQ, NK], I32, kind="ExternalInput")
    wqT = nc.dram_tensor("wqT", [D, D], F32, kind="ExternalInput")
    wkT = nc.dram_tensor("wkT", [D, D], F32, kind="ExternalInput")
    out = nc.dram_tensor("out", [BPC, NQ, NK], F32, kind="ExternalOutput")

    with TileContext(nc) as tc:
        with (
            tc.tile_pool(name="const", bufs=1) as cp,
            tc.tile_pool(name="batch", bufs=2) as bp,
            tc.tile_pool(name="mt", bufs=4) as mp,
            tc.tile_pool(name="ps", bufs=2, space="PSUM") as ps,
        ):
            # ---- A = Wq @ Wk^T, once ----
            wq_t = cp.tile([128, 2, D], F32)   # [e-part, e-chunk, d]
            wk_t = cp.tile([128, 2, D], F32)
            nc.sync.dma_start(out=wq_t[:], in_=wqT[:, :].rearrange("(c p) d -> p c d", p=128))
            nc.sync.dma_start(out=wk_t[:], in_=wkT[:, :].rearrange("(c p) d -> p c d", p=128))
            a_sb = []                          # a_sb[dc][:, dp] = A[dc*128+:, :]
            for dc in range(2):
                a_ps = ps.tile([128, D], F32, tag="sc", name=f"a_ps{dc}")
                for ec in range(2):
                    nc.tensor.matmul(
                        a_ps[:],
                        wq_t[:, ec, dc * 128:(dc + 1) * 128],
                        wk_t[:, ec, :],
                        start=(ec == 0),
                        stop=(ec == 1),
                    )
                a_t = cp.tile([128, D], F32, name=f"a_t{dc}")
                nc.vector.tensor_copy(a_t[:], a_ps[:])
                a_sb.append(a_t)

            ebias = cp.tile([128, 1], F32)
            nc.vector.memset(ebias[:], -10.0)

            for b in [bb for _ in range(reps) for bb in range(BPC)]:
                # ---- load this batch's operands ----
                qT_t = bp.tile([128, 2, NQ], F32)
                kT_t = bp.tile([128, 2, NK], F32, bufs=1)   # staging for the f32r round
                kTr_t = bp.tile([128, 2, NK], F32R)
                for dc in range(2):
                    nc.sync.dma_start(out=qT_t[:, dc], in_=qT[b, dc * 128:(dc + 1) * 128, :])
                    nc.sync.dma_start(out=kT_t[:, dc], in_=kT[b, dc * 128:(dc + 1) * 128, :])
                    nc.vector.tensor_copy(kTr_t[:, dc], kT_t[:, dc])  # round to f32r
                kTr = kTr_t[:]

                # ---- qaT[d', m] = sum_d A[d, d'] q_inT[d, m]  (fp32) ----
                qa_t = bp.tile([128, 2, NQ], F32R)
                for dp in range(2):
                    for mc in range(NCH):
                        qa_ps = ps.tile([128, CH], F32, tag="sc", name="qa_ps")
                        for dc in range(2):
                            nc.tensor.matmul(
                                qa_ps[:],
                                a_sb[dc][:, dp * 128:(dp + 1) * 128],
                                qT_t[:, dc, mc * CH:(mc + 1) * CH],
                                start=(dc == 0),
                                stop=(dc == 1),
                            )
                        nc.vector.tensor_copy(qa_t[:, dp, mc * CH:(mc + 1) * CH], qa_ps[:])

                # ---- per m-tile: scores -> tanh -> +adj -> exp+sum -> norm ----
                for mt in range(NMT):
                    sc_ps = ps.tile([128, NK], F32, tag="sc", name="sc_ps")
                    for dp in range(2):
                        for n in range(NCH):
                            nc.tensor.matmul(
                                sc_ps[:, n * CH:(n + 1) * CH],
                                qa_t[:, dp, mt * MT:(mt + 1) * MT],
                                kTr[:, dp, n * CH:(n + 1) * CH],
                                start=(dp == 0),
                                stop=(dp == 1),
                            )
                    adj_t = mp.tile([128, NK], I32)
                    nc.sync.dma_start(out=adj_t[:], in_=adj[b, mt * MT:(mt + 1) * MT, :])
                    t_t = mp.tile([128, NK], F32)
                    nc.scalar.activation(t_t[:], sc_ps[:], AF.Tanh, scale=1.0 / 16.0)
                    nc.gpsimd.tensor_add(t_t[:], t_t[:], adj_t[:])
                    rsum = mp.tile([128, 1], F32, bufs=2)
                    nc.scalar.activation(t_t[:], t_t[:], AF.Exp, bias=ebias[:], scale=10.0, accum_out=rsum[:])
                    rcp = mp.tile([128, 1], F32, bufs=2)
                    nc.vector.reciprocal(rcp[:], rsum[:])
                    nc.vector.tensor_scalar_mul(t_t[:], t_t[:], rcp[:])
                    nc.sync.dma_start(out=out[b, mt * MT:(mt + 1) * MT, :], in_=t_t[:])
    nc.compile()
    return nc


_NC = None


def _get_nc():
    global _NC
    if _NC is None:
        _NC = build()
    return _NC


def kernel(k_inputs, q_inputs, adjancy, Wk, Wq):
    k_inputs = np.asarray(k_inputs, dtype=np.float32)
    q_inputs = np.asarray(q_inputs, dtype=np.float32)
    adjancy = np.asarray(adjancy, dtype=np.int32)
    Wk = np.asarray(Wk, dtype=np.float32)
    Wq = np.asarray(Wq, dtype=np.float32)
    nc = _get_nc()
    wqT = np.ascontiguousarray(Wq.T)
    wkT = np.ascontiguousarray(Wk.T)
    in_maps = []
    for c in range(CORES):
        lo, hi = c * BPC, (c + 1) * BPC
        in_maps.append({
            "qT": np.ascontiguousarray(q_inputs[lo:hi].transpose(0, 2, 1)),
            "kT": np.ascontiguousarray(k_inputs[lo:hi].transpose(0, 2, 1)),
            "adj": np.ascontiguousarray(adjancy[lo:hi]),
            "wqT": wqT,
            "wkT": wkT,
        })
    res = run_bass_kernel_spmd(nc, in_maps, core_ids=list(range(CORES)))
    return np.concatenate([res.results[c]["out"] for c in range(CORES)], axis=0)


# revision 13
# speedup vs baseline: 2.2120x; 1.0166x over previous
"""Trainium2 Bass kernel for masked tanh-clipped attention softmax.

Reference computation (B=16, NQ=NK=2048, KD=QD=KQ=256, CLIP=10):
    k = k_inputs @ Wk                     [B, NK, 256]
    q = q_inputs @ Wq                     [B, NQ, 256]
    s = (q @ k^T) / 16                    [B, NQ, NK]
    s = tanh(s) * 10
    s = where(adjancy, s, -inf)
    out = softmax(s, axis=2)

Kernel strategy (per NeuronCore, 2 batches each across 8 cores):
  * Fold the projections: A = Wq @ Wk^T (256x256), so s = q_in @ A @ k_in^T.
    Computed once on device in fp32.
  * Host passes q_in/k_in pre-transposed to [d, token] layout (the PE
    contracts over the partition dim), plus Wq^T/Wk^T for the A matmul.
  * qaT = A^T @ q_inT per batch in fp32 (PE), rounded to f32r on copy-out.
  * scores = qaT^T @ k_inT in f32r (full PE speed, ~1e-4 rel err).
  * t = tanh(scores/16) on ACT (PSUM -> SBUF).
  * t += adjancy (int32 0/1) on GpSimd.
  * e = exp(10t - 10) on ACT with fused row-sum (accum_out).  Unmasked
    entries give exp(10*tanh) exactly; masked ones are suppressed by e^-10,
    contributing < 5e-5 relative to the row sum (reference gives exact 0).
    No max-subtraction needed: scores are clipped to [-10, 10].
  * out = e * (1/rowsum) on DVE (per-partition tensor_scalar).
"""
import numpy as np

import concourse.bacc as bacc
import concourse.mybir as mybir
from concourse.tile import TileContext
from concourse.bass_utils import run_bass_kernel_spmd

F32 = mybir.dt.float32
F32R = mybir.dt.float32r
I32 = mybir.dt.int32
AF = mybir.ActivationFunctionType

B, NQ, NK = 16, 2048, 2048
D = 256                 # KD = QD = KQ
CORES = 8
BPC = B // CORES        # batches per core
MT = 128                # query rows per tile
NMT = NQ // MT          # 16 m-tiles per batch
CH = 512                # psum bank free-dim (fp32)
NCH = NK // CH          # 4 n-chunks per scores row


def build(reps=1):
    nc = bacc.Bacc(None, target_bir_lowering=False)

    qT = nc.dram_tensor("qT", [BPC, D, NQ], F32, kind="ExternalInput")
    kT = nc.dram_tensor("kT", [BPC, D, NK], F32, kind="ExternalInput")
    adj = nc.dram_tensor("adj", [BPC, NQ, NK], I32, kind="ExternalInput")
    wqT = nc.dram_tensor("wqT", [D, D], F32, kind="ExternalInput")
    wkT = nc.dram_tensor("wkT", [D, D], F32, kind="ExternalInput")
    out = nc.dram_tensor("out", [BPC, NQ, NK], F32, kind="ExternalOutput")

    with TileContext(nc) as tc:
        with (
            tc.tile_pool(name="const", bufs=1) as cp,
            tc.tile_pool(name="batch", bufs=2) as bp,
            tc.tile_pool(name="mt", bufs=4) as mp,
            tc.tile_pool(name="ps", bufs=2, space="PSUM") as ps,
        ):
            # ---- A = Wq @ Wk^T, once ----
            wq_t = cp.tile([128, 2, D], F32)   # [e-part, e-chunk, d]
            wk_t = cp.tile([128, 2, D], F32)
            nc.sync.dma_start(out=wq_t[:], in_=wqT[:, :].rearrange("(c p) d -> p c d", p=128))
            nc.sync.dma_start(out=wk_t[:], in_=wkT[:, :].rearrange("(c p) d -> p c d", p=128))
            a_sb = []                          # a_sb[dc][:, dp] = A[dc*128+:, :]
            for dc in range(2):
                a_ps = ps.tile([128, D], F32, tag="sc", name=f"a_ps{dc}")
                for ec in range(2):
                    nc.tensor.matmul(
                        a_ps[:],
                        wq_t[:, ec, dc * 128:(dc + 1) * 128],
                        wk_t[:, ec, :],
                        start=(ec == 0),
                        stop=(ec == 1),
                    )
                a_t = cp.tile([128, D], F32, name=f"a_t{dc}")
                nc.vector.tensor_copy(a_t[:], a_ps[:])
                a_sb.append(a_t)

            ebias = cp.tile([128, 1], F32)
            nc.vector.memset(ebias[:], -10.0)

            for b in [bb for _ in range(reps) for bb in range(BPC)]:
                # ---- load this batch's operands ----
                qT_t = bp.tile([128, 2, NQ], F32)
                kT_t = bp.tile([128, 2, NK], F32, bufs=1)   # staging for the f32r round
                kTr_t = bp.tile([128, 2, NK], F32R)
                for dc in range(2):
                    nc.sync.dma_start(out=qT_t[:, dc], in_=qT[b, dc * 128:(dc + 1) * 128, :])
                    nc.sync.dma_start(out=kT_t[:, dc], in_=kT[b, dc * 128:(dc + 1) * 128, :])
                    nc.vector.tensor_copy(kTr_t[:, dc], kT_t[:, dc])  # round to f32r
                kTr = kTr_t[:]

                # ---- qaT[d', m] = sum_d A[d, d'] q_inT[d, m]  (fp32) ----
                qa_t = bp.tile([128, 2, NQ], F32R)
                for dp in range(2):
                    for mc in range(NCH):
                        qa_ps = ps.tile([128, CH], F32, tag="sc", name="qa_ps")
                        for dc in range(2):
                            nc.tensor.matmul(
                                qa_ps[:],
                                a_sb[dc][:, dp * 128:(dp + 1) * 128],
                                qT_t[:, dc, mc * CH:(mc + 1) * CH],
                                start=(dc == 0),
                                stop=(dc == 1),
                            )
                        nc.vector.tensor_copy(qa_t[:, dp, mc * CH:(mc + 1) * CH], qa_ps[:])

                # ---- per m-tile: scores -> tanh -> +adj -> exp+sum -> norm ----
                for mt in range(NMT):
                    sc_ps = ps.tile([128, NK], F32, tag="sc", name="sc_ps")
                    for dp in range(2):
                        for n in range(NCH):
                            nc.tensor.matmul(
                                sc_ps[:, n * CH:(n + 1) * CH],
                                qa_t[:, dp, mt * MT:(mt + 1) * MT],
                                kTr[:, dp, n * CH:(n + 1) * CH],
                                start=(dp == 0),
                                stop=(dp == 1),
                            )
                    adj_t = mp.tile([128, NK], I32)
                    nc.sync.dma_start(out=adj_t[:], in_=adj[b, mt * MT:(mt + 1) * MT, :])
                    t_t = mp.tile([128, NK], F32)
                    nc.scalar.activation(t_t[:], sc_ps[:], AF.Tanh, scale=1.0 / 16.0)
                    nc.gpsimd.tensor_add(t_t[:], t_t[:], adj_t[:])
                    rsum = mp.tile([128, 1], F32, bufs=2)
                    nc.scalar.activation(t_t[:], t_t[:], AF.Exp, bias=ebias[:], scale=10.0, accum_out=rsum[:])
                    rcp = mp.tile([128, 1], F32, bufs=2)
                    nc.vector.reciprocal(rcp[:], rsum[:])
                    nc.vector.tensor_scalar_mul(t_t[:], t_t[:], rcp[:])
                    nc.sync.dma_start(out=out[b, mt * MT:(mt + 1) * MT, :], in_=t_t[:])
    nc.compile()
    return nc


_NC = None


def _get_nc():
    global _NC
    if _NC is None:
        _NC = build()
    return _NC


def kernel(k_inputs, q_inputs, adjancy, Wk, Wq):
    k_inputs = np.asarray(k_inputs, dtype=np.float32)
    q_inputs = np.asarray(q_inputs, dtype=np.float32)
    adjancy = np.asarray(adjancy, dtype=np.int32)
    Wk = np.asarray(Wk, dtype=np.float32)
    Wq = np.asarray(Wq, dtype=np.float32)
    nc = _get_nc()
    wqT = np.ascontiguousarray(Wq.T)
    wkT = np.ascontiguousarray(Wk.T)
    in_maps = []
    for c in range(CORES):
        lo, hi = c * BPC, (c + 1) * BPC
        in_maps.append({
            "qT": np.ascontiguousarray(q_inputs[lo:hi].transpose(0, 2, 1)),
            "kT": np.ascontiguousarray(k_inputs[lo:hi].transpose(0, 2, 1)),
            "adj": np.ascontiguousarray(adjancy[lo:hi]),
            "wqT": wqT,
            "wkT": wkT,
        })
    res = run_bass_kernel_spmd(nc, in_maps, core_ids=list(range(CORES)))
    return np.concatenate([res.results[c]["out"] for c in range(CORES)], axis=0)
